# revision 41
# baseline (speedup 1.0000x reference)
"""Differential attention kernel for Trainium2 (8 NeuronCores, SPMD).

Math per (batch, head):
    q1,q2 / k1,k2 = halves of head_dim (D=64 -> d2=32)
    a_i = softmax(q_i @ k_i^T / sqrt(d2))        (i = 1,2)
    out = (a1 - lam*a2) @ V, then per-(q) groupnorm over D, scaled by (1-0.8).

Design (per core: 4 of the 32 (b,h) pairs). The kernel is ScalarE-bound
(~34M exp evaluations/core at 128 lanes x 1.2 GHz ~= 219us pure), so the
structure keeps ScalarE saturated, offloads a slice of the exp work to the
DVE, and keeps everything else off the critical path.
  - Q/K/V cast to fp16 on host (single-pass PE matmul). Host ships Q^T/K^T
    twice ([2D, s]) so 4 score matmuls (K=32) run concurrently in the four
    32-row PE row groups via tile_position.
  - Scores computed transposed: S^T[k, q] units [128, 512] packed into
    alternating 4-bank/2-bank PSUM tiles; each tile drained by ONE ScalarE
    exp (FD 2048/1024) to amortize the ~293ns per-instruction overhead.
    exp needs no max-subtraction: scores ~ N(0,1).
  - Tag patterns alternate by chunk parity so every same-tag PSUM reuse is
    >= 2 groups apart, and score groups are emitted LOOKAHEAD groups ahead
    of the AV stream, so a freed PSUM tag's refill is already at the front
    of the PE queue -- otherwise the exp -> refill -> exp tag round-trip
    paces the pipeline (~3.5us/chunk slower).
    (An X_DVE path that offloads units to a DVE bit-splice exp2 exists but
    is off: every placement ran into the ~3us single-buffered-tag reuse
    deadline vs the DVE op-chain latency and lost 30-100us.)
  - U^T = exp(S^T) fp16 in SBUF; AV matmul lhsT = [V | ones] (M=65) so row
    64 accumulates softmax row-sums r for free. AV chunks pipeline in one
    flat (head, q-chunk) stream across head boundaries.
  - O^T[65, q] per 512-q chunk is copied fp16 to SBUF (pad rows zeroed once
    in static staging tiles); DMA xbar transpose flips [80, 512] ->
    [128, 4, 80] (col 64 of each block = r).
  - Combine exploits scale invariance of groupnorm: instead of
    W = O1*r1inv - lam*O2*r2inv, compute W'' = O1u - (lam*r1/r2)*O2u = r1*W
    (one fused scalar_tensor_tensor per q-tile; batched reciprocals). The
    only difference vs the reference is EPS seen as EPS/r1^2 ~ 0 inside
    rsqrt(var+EPS) -- a ~2e-3 relative effect since var >> EPS here.
  - rstd = (1-lam0)*rsqrt(var+EPS): integer-magic seed on the DVE, then the
    Newton iteration and the final (W - mean)*rstd rescale run on GPSIMD
    (otherwise idle, own queue) -- keeping the DVE FIFO clear is critical,
    since the AV PSUM copy-outs gate the next pav accumulation through the
    PE's in-order queue. The last head finishes incrementally, its final
    q-tiles via ScalarE ln/exp (table switch is free once exps are done),
    so the post-loop tail stays short. A strided store DMA per range.
"""

import math
import numpy as np

import concourse.bass as bass
import concourse.tile as tile
from concourse import bacc, mybir
from concourse.bass_utils import run_bass_kernel_spmd

F32 = mybir.dt.float32
F16 = mybir.dt.float16
I32 = mybir.dt.int32
AF = mybir.ActivationFunctionType
ALU = mybir.AluOpType

B, H, S, D = 2, 16, 2048, 64
D2 = D // 2
N_CORES = 8
HEADS_PER_CORE = (B * H) // N_CORES  # 4
LAMBDA_INIT = 0.8
EPS = 1e-5
SCALE = 1.0 / math.sqrt(D2)
LOG2E = math.log2(math.e)

QC = 512           # q chunk (one PSUM bank of fp32)
KTILE = 128        # k tile (partition dim)
OROWS = 80         # O^T rows padded to xbar 16-row granularity (65 -> 80)

# --- tunables ---
X_DVE = 0          # score units per chunk exp'd on the DVE
AV_BUFS = 2
GP_FINISH = False   # final rescale on GPSIMD instead of DVE
SC_RSQRT = False    # rstd via ScalarE ln/exp instead of DVE Newton
U_BUFS = 30
LOOKAHEAD = 4      # score groups emitted ahead of the AV stream (PE queue
                   # position of a PSUM-tag refill when ScalarE frees it)
NEWTON_ITERS = 4   # rsqrt Newton steps (err 1.5e^2 per step)
TAIL_SC_RSQRT = True  # last head's rsqrt on the (by then idle) ScalarE

# 2^f quadratic on [-0.5, 0.5], minimax relative error ~1.7e-3
EXP2_C0 = 1.0004428354418449
EXP2_C1 = 0.7034397194807023
EXP2_C2 = 0.23842415021588936
MAGIC = float(1.5 * 2 ** 23)   # fp32 round-to-int magic


def chunk_plan(nkt, x_dve):
    """Per-chunk-parity group plans. Each group is one PSUM tile (ps4 = 4
    banks / ps2 = 2 banks) filled by the PE; groups with sc units are
    drained by one ScalarE exp, dve units by the DVE's bit-splice exp2.
    Tag patterns alternate by chunk parity so every same-tag reuse
    (including across chunk boundaries) is >= 2 groups apart -- a 1-group
    reuse serializes PE fill against ScalarE drain once per chunk. The
    dve-only group sits mid-chunk, away from the end-of-chunk norm blob,
    so the DVE drains its tile within the 2-group reuse deadline."""
    dve_units = [(nkt - 1 - i, 1) for i in range(x_dve)]  # a2 half
    sc_units = [(kt, h) for kt in range(nkt) for h in (0, 1)
                if (kt, h) not in dve_units]
    plans = []
    for par in range(2):
        if x_dve == 0:
            if par == 0:
                sizes = [4, 2, 4, 2, 4, 2, 4, 2, 4, 2, 2]
                tags = ["ps4", "ps2"] * 5 + ["ps4"]
            else:
                sizes = [2, 4, 2, 4, 2, 4, 2, 4, 2, 4, 2]
                tags = ["ps2", "ps4"] * 5 + ["ps2"]
            dve_at = None
        elif x_dve == 2:
            if par == 0:
                sizes = [4, 2, 4, 2, 4, 0, 4, 2, 4, 2, 2]
                tags = ["ps4", "ps2"] * 5 + ["ps4"]
                dve_at = 5
            else:
                sizes = [2, 4, 2, 4, 0, 4, 2, 4, 2, 4, 2]
                tags = ["ps2", "ps4"] * 5 + ["ps2"]
                dve_at = 4
        else:
            raise ValueError(x_dve)
        assert sum(sizes) == 2 * nkt - x_dve
        groups = []
        pos = 0
        for n in sizes:
            groups.append(sc_units[pos:pos + n])
            pos += n
        dve_in = {dve_at: list(dve_units)} if dve_at is not None else {}
        plans.append((groups, tags, dve_in))
    ngrp = len(plans[0][0])
    def mk_ranges(seq):
        return [seq[round(2 * nkt * g / ngrp):round(2 * nkt * (g + 1) / ngrp)]
                for g in range(ngrp)]
    m_ranges = mk_ranges(
        [kt + half * nkt for kt in range(nkt) for half in (0, 1)])
    m_ranges_il = m_ranges
    # verify all same-tag reuse distances >= 2 across the alternation
    seq = [t for par in (0, 1) for t in plans[par][1]] * 2
    for i in range(len(seq)):
        for j in range(i + 1, len(seq)):
            if seq[j] == seq[i]:
                assert j - i >= 2, (i, j)
                break
    return plans, m_ranges, m_ranges_il


def build_program(n_heads=HEADS_PER_CORE, s=S):
    nq = s // QC          # q chunks per head
    nkt = s // KTILE      # k tiles per head
    nqt = s // 128        # q tiles (norm phase)
    tpq = QC // 128       # q tiles per chunk
    plans, m_ranges, m_ranges_il = chunk_plan(nkt, X_DVE)
    ngrp = len(plans[0][0])

    nc = bacc.Bacc("TRN2", target_bir_lowering=False, debug=False,
                   num_devices=N_CORES)
    qt_d = nc.dram_tensor("qt", [n_heads, 2 * D, s], F16, kind="ExternalInput")
    kt_d = nc.dram_tensor("kt", [n_heads, 2 * D, s], F16, kind="ExternalInput")
    v_d = nc.dram_tensor("v", [n_heads, s, D], F16, kind="ExternalInput")
    lam_d = nc.dram_tensor("lam", [n_heads, 1], F32, kind="ExternalInput")
    out_d = nc.dram_tensor("out", [n_heads, s, D], F32, kind="ExternalOutput")

    gp = nc.gpsimd

    with tile.TileContext(nc) as tc:
        with (
            tc.tile_pool(name="consts", bufs=1) as consts,
            tc.tile_pool(name="qk", bufs=2) as qk_pool,
            tc.tile_pool(name="vx", bufs=2) as vx_pool,
            tc.tile_pool(name="lamp", bufs=2) as lam_pool,
            tc.tile_pool(name="u", bufs=U_BUFS) as u_pool,
            tc.tile_pool(name="ud", bufs=2 * X_DVE + 2) as ud_pool,
            tc.tile_pool(name="dv", bufs=2) as dv_pool,
            tc.tile_pool(name="o", bufs=1) as o_pool,
            tc.tile_pool(name="tr", bufs=6) as tr_pool,
            tc.tile_pool(name="w", bufs=3) as w_pool,
            tc.tile_pool(name="stats", bufs=3) as stats_pool,
            tc.tile_pool(name="small", bufs=8) as small_pool,
            tc.tile_pool(name="ps_sc", bufs=1, space="PSUM") as ps_scores,
            tc.tile_pool(name="ps_av", bufs=AV_BUFS, space="PSUM") as ps_av,
        ):
            eps_ap = consts.tile([128, 1], F32)
            nc.vector.memset(eps_ap, EPS)
            bexp_ap = consts.tile([128, 1], F32)
            nc.vector.memset(bexp_ap, math.log(1.0 - LAMBDA_INIT))
            zero_ap = consts.tile([128, 1], F32)
            nc.vector.memset(zero_ap, 0.0)
            consts_c15 = consts.tile([128, 16], F32)
            nc.vector.memset(consts_c15, 1.5)
            consts_cfin = consts.tile([128, 16], F32)
            nc.vector.memset(consts_cfin,
                             (1.0 - LAMBDA_INIT) / math.sqrt(2.0))
            # prefetch the exp table set while the first DMAs run
            warm = consts.tile([128, 1], F32)
            if SC_RSQRT:
                nc.scalar.activation(warm, eps_ap, AF.Ln, bias=eps_ap)
                nc.scalar.activation(warm, warm, AF.Exp)
            else:
                nc.scalar.activation(warm, eps_ap, AF.Exp)

            # Two static sets of O^T staging tiles (heads alternate) so the
            # xbar-transpose pad rows 65..79 are zeroed exactly once.
            o_static = []
            for par in range(2):
                o1s = o_pool.tile([OROWS, s], F16, tag=f"o1_{par}")
                o2s = o_pool.tile([OROWS, s], F16, tag=f"o2_{par}")
                nc.gpsimd.memset(o1s[D:OROWS, :], 0.0)
                nc.gpsimd.memset(o2s[D:OROWS, :], 0.0)
                o_static.append((o1s, o2s))

            head_state = {}
            pending_ops = []  # (tag, fn) FIFO; tag marks a DVE-exp unit

            def pump(n):
                for _ in range(min(n, len(pending_ops))):
                    pending_ops.pop(0)[1]()

            def pump_until_emitted(tag):
                """Emit queued ops up to and including the last op of `tag`
                (its consumer is about to be emitted)."""
                while any(t == tag for t, _ in pending_ops):
                    pending_ops.pop(0)[1]()

            def load_head(h):
                qt_sb = qk_pool.tile([2 * D, s], F16, tag="qt")
                kt_sb = qk_pool.tile([2 * D, s], F16, tag="kt")
                if h == 0:
                    # finely chunked so the first score group starts early
                    for tsb, td, c0, c1 in (
                        (kt_sb, kt_d, 0, 128), (qt_sb, qt_d, 0, 512),
                        (kt_sb, kt_d, 128, 512), (kt_sb, kt_d, 512, 1024),
                        (kt_sb, kt_d, 1024, 2048), (qt_sb, qt_d, 512, 2048),
                    ):
                        nc.sync.dma_start(out=tsb[:, c0:c1],
                                          in_=td[h][:, c0:c1])
                else:
                    nc.sync.dma_start(out=qt_sb, in_=qt_d[h])
                    nc.sync.dma_start(out=kt_sb, in_=kt_d[h])
                vx = vx_pool.tile([128, nkt, D + 1], F16, tag="vx")
                nc.sync.dma_start(
                    out=vx[:, :, 0:D],
                    in_=v_d[h].rearrange("(t p) d -> p t d", p=128),
                )
                nc.vector.memset(vx[:, :, D : D + 1], 1.0)
                lamneg = lam_pool.tile([128, 1], F32, tag="lam")
                nc.sync.dma_start(out=lamneg, in_=lam_d[h].to_broadcast((128, 1)))
                nc.vector.tensor_scalar_mul(lamneg, lamneg, -1.0)
                o1, o2 = o_static[h % 2]
                w_head = w_pool.tile([128, nqt, D], F32, tag="w")
                head_state[h] = dict(qt=qt_sb, kt=kt_sb, vx=vx, lamneg=lamneg,
                                     o1=o1, o2=o2, w=w_head, mv=None)

            def queue_dve_exp(h, qc, kt, half, ps_src):
                """One score unit exp'd on the DVE from a PSUM slice: exact
                2^n bit-splice times quadratic 2^f. Ops are queued and
                dribbled so they never form a DVE-FIFO block ahead of the
                AV copy-outs."""
                st = head_state[h]
                ud = ud_pool.tile([128, QC], F16, tag="ud")
                bu = dv_pool.tile([128, QC], F32, tag="bu")
                bn = dv_pool.tile([128, QC], F32, tag="bn")
                be = dv_pool.tile([128, QC], F32, tag="be")
                bf = dv_pool.tile([128, QC], F32, tag="bf")
                bp = dv_pool.tile([128, QC], F32, tag="bp")
                a = SCALE * LOG2E
                for f in [
                    # u = t*a + MAGIC  (rounds t*a to int in low mantissa)
                    lambda: nc.vector.tensor_scalar(
                        out=bu, in0=ps_src, scalar1=a, scalar2=MAGIC,
                        op0=ALU.mult, op1=ALU.add),
                    # n = u - MAGIC (exact)
                    lambda: nc.vector.tensor_scalar(
                        out=bn, in0=bu, scalar1=MAGIC, scalar2=None,
                        op0=ALU.subtract),
                    # 2^n bits = (u_bits << 23) + (127 << 23); walrus forbids
                    # mixing bitwise and arith op classes in one instruction
                    lambda: nc.vector.tensor_scalar(
                        out=be[:].bitcast(I32), in0=bu[:].bitcast(I32),
                        scalar1=23, scalar2=None,
                        op0=ALU.logical_shift_left),
                    lambda: nc.vector.tensor_scalar(
                        out=be[:].bitcast(I32), in0=be[:].bitcast(I32),
                        scalar1=(127 << 23), scalar2=None, op0=ALU.add),
                    # f = t*a - n  in [-0.5, 0.5]
                    lambda: nc.vector.scalar_tensor_tensor(
                        out=bf, in0=ps_src, scalar=a, in1=bn,
                        op0=ALU.mult, op1=ALU.subtract),
                    # p = c2*f + c1
                    lambda: nc.vector.tensor_scalar(
                        out=bp, in0=bf, scalar1=EXP2_C2, scalar2=EXP2_C1,
                        op0=ALU.mult, op1=ALU.add),
                    # p = p*f
                    lambda: nc.vector.tensor_mul(bp, bp, bf),
                    # u_out = (p + c0) * 2^n
                    lambda: nc.vector.scalar_tensor_tensor(
                        out=ud, in0=bp, scalar=EXP2_C0, in1=be,
                        op0=ALU.add, op1=ALU.mult),
                ]:
                    f()
                st[("umap", qc)][half * nkt + kt] = ud[:, :]

            def score_group(h, qc, g):
                """One group of score units -> one PSUM tile -> one exp
                (or, for the dve-only group, eight queued DVE ops)."""
                st = head_state[h]
                qt_sb, kt_sb = st["qt"], st["kt"]
                groups, tags, dve_in = plans[(h * nq + qc + 1) % 2]
                units = groups[g]
                dunits = dve_in.get(g, ())
                n = len(units)
                nb = 4 if tags[g] == "ps4" else 2
                ps = ps_scores.tile([128, nb * QC], F32, tag=tags[g])
                for i, (kt, half) in enumerate(list(units) + list(dunits)):
                    j = 2 * kt + half
                    rb = (j % 4) * D2
                    dsl = slice(rb, rb + D2)
                    nc.tensor.matmul(
                        ps[:, i * QC : (i + 1) * QC],
                        kt_sb[dsl, kt * KTILE : (kt + 1) * KTILE],
                        qt_sb[dsl, qc * QC : (qc + 1) * QC],
                        start=True, stop=True,
                        tile_position=(rb, 0),
                    )
                if n:
                    u = u_pool.tile([128, 4 * QC], F16, tag="u")
                    nc.scalar.activation(
                        u[:, 0 : n * QC], ps[:, 0 : n * QC], AF.Exp,
                        scale=SCALE)
                    for i, (kt, half) in enumerate(units):
                        st[("umap", qc)][half * nkt + kt] = (
                            u[:, i * QC : (i + 1) * QC])
                for i, (kt, half) in enumerate(dunits):
                    sl = slice((n + i) * QC, (n + i + 1) * QC)
                    queue_dve_exp(h, qc, kt, half, ps[:, sl])

            def av_chunk(h, qc, ms):
                """AV matmuls for m in ms; m = half*nkt + kt."""
                st = head_state[h]
                vx = st["vx"]
                umap = st[("umap", qc)]
                for m in ms:
                    half, kt = divmod(m, nkt)
                    if kt == 0:
                        pav_new = ps_av.tile([D + 1, QC], F32, tag="pav")
                        st[("pav", qc, half)] = pav_new
                    pav = st[("pav", qc, half)]
                    nc.tensor.matmul(
                        pav, vx[:, kt, :], umap.pop(m),
                        start=(kt == 0), stop=(kt == nkt - 1),
                    )
                    if kt == nkt - 1:
                        o_sb = st["o2"] if half else st["o1"]
                        nc.vector.tensor_copy(
                            o_sb[0 : D + 1, qc * QC : (qc + 1) * QC], pav)
                        del st[("pav", qc, half)]

            def norm_chunk(h, qc):
                """DMA-xbar transpose of one q chunk, scale-free combine,
                stats. W'' = O1u - (lam*r1/r2)*O2u = r1*W; groupnorm is
                invariant to the per-row r1 factor (up to EPS/r1^2 ~ 0)."""
                st = head_state[h]
                o1, o2, lamneg = st["o1"], st["o2"], st["lamneg"]
                if st["mv"] is None:
                    mv_new = stats_pool.tile([128, nqt, 2], F32, tag="mv")
                    st["mv"] = mv_new
                mv = st["mv"]
                csl = slice(qc * QC, (qc + 1) * QC)
                tr1 = tr_pool.tile([128, tpq, OROWS], F16, tag="tr1")
                nc.sync.dma_start_transpose(tr1, o1[:, csl])
                tr2 = tr_pool.tile([128, tpq, OROWS], F16, tag="tr2")
                nc.sync.dma_start_transpose(tr2, o2[:, csl])
                rho = small_pool.tile([128, tpq], F32, tag="rho")
                nc.vector.reciprocal(rho, tr2[:, :, D])
                # rho = -lam * r1 / r2
                nc.vector.tensor_mul(rho, rho, tr1[:, :, D])
                nc.vector.tensor_scalar_mul(rho, rho, lamneg)
                for t in range(tpq):
                    qt_i = qc * tpq + t
                    w = st["w"][:, qt_i, :]
                    nc.vector.scalar_tensor_tensor(
                        out=w, in0=tr2[:, t, 0:D],
                        scalar=rho[:, t : t + 1], in1=tr1[:, t, 0:D],
                        op0=ALU.mult, op1=ALU.add,
                    )
                    s6 = small_pool.tile([128, 6], F32, tag="s6")
                    nc.vector.bn_stats(out=s6, in_=w)
                    nc.vector.bn_aggr(out=mv[:, qt_i, :], in_=s6)

            def finish_head_a(h, q0, q1, on_scalar=False):
                """y[:, q0:q1] = (1-lam0) * rsqrt(var + EPS): either two
                ScalarE activations exp(-0.5*ln(var+EPS) + ln(1-lam0)) --
                used only when ScalarE has gone idle (tail), since Ln lives
                in a different table set than Exp -- or a DVE Newton
                iteration with an exponent-only integer seed, dribbled via
                the pending queue."""
                st = head_state[h]
                mv = st["mv"]
                if "y" not in st:
                    y_new = stats_pool.tile([128, nqt], F32, tag="y")
                    st["y"] = y_new
                y = st["y"]
                sl = slice(q0, q1)
                if on_scalar:
                    lnv = stats_pool.tile([128, nqt], F32, tag="lnv")
                    yt = stats_pool.tile([128, nqt], F32, tag="yt")
                    st["y"] = yt
                    nc.scalar.activation(lnv[:, sl], mv[:, sl, 1], AF.Ln,
                                         bias=eps_ap)
                    nc.scalar.activation(yt[:, sl], lnv[:, sl], AF.Exp,
                                         bias=bexp_ap, scale=-0.5)
                    return
                # DVE Newton rsqrt (exponent seed + NEWTON_ITERS steps)
                x = stats_pool.tile([128, nqt], F32, tag="x")
                t = stats_pool.tile([128, nqt], F32, tag="t")
                c15 = consts_c15
                cfin = consts_cfin
                xs, ys, ts = x[:, sl], y[:, sl], t[:, sl]
                ops = [
                    lambda: nc.vector.tensor_scalar(
                        out=xs, in0=mv[:, sl, 1], scalar1=EPS,
                        scalar2=0.25, op0=ALU.add, op1=ALU.mult),
                    lambda: nc.vector.tensor_scalar(
                        out=ys.bitcast(I32), in0=xs.bitcast(I32),
                        scalar1=23, scalar2=None,
                        op0=ALU.logical_shift_right),
                    lambda: nc.vector.tensor_scalar(
                        out=ys.bitcast(I32), in0=ys.bitcast(I32),
                        scalar1=-1, scalar2=380, op0=ALU.mult, op1=ALU.add),
                    lambda: nc.vector.tensor_scalar(
                        out=ys.bitcast(I32), in0=ys.bitcast(I32),
                        scalar1=1, scalar2=None,
                        op0=ALU.logical_shift_right),
                    lambda: nc.vector.tensor_scalar(
                        out=ys.bitcast(I32), in0=ys.bitcast(I32),
                        scalar1=23, scalar2=None,
                        op0=ALU.logical_shift_left),
                ]
                for _ in range(NEWTON_ITERS):
                    ops += [
                        lambda: gp.tensor_mul(ts, ys, ys),
                        lambda: gp.tensor_mul(ts, ts, xs),
                        lambda: gp.tensor_sub(ts, c15[:, sl], ts),
                        lambda: gp.tensor_mul(ys, ys, ts),
                    ]
                ops.append(lambda: gp.tensor_mul(ys, ys, cfin[:, sl]))
                pending_ops.extend((None, f) for f in ops)

            def finish_head_b(h, q0, q1, pop=True, eng=None):
                st = head_state[h]
                mv, y = st["mv"], st["y"]
                if eng is not None or "m2" not in st:
                    tag2 = "m2t" if eng is not None else "m2"
                    m2_new = stats_pool.tile([128, nqt], F32, tag=tag2)
                    st["m2"] = m2_new
                m2 = st["m2"]
                nq_ = q1 - q0
                sl = slice(q0, q1)
                w3 = st["w"][:, sl, :]
                yb = y[:, sl][:, :, None].broadcast_to((128, nq_, D))
                m2b = m2[:, sl][:, :, None].broadcast_to((128, nq_, D))
                wd = out_d[h].rearrange("(t p) d -> p t d", p=128)[:, sl, :]
                eng_dma = nc.sync if eng is not None else gp
                eng = eng or gp
                ops = [
                    lambda: eng.tensor_mul(m2[:, sl], mv[:, sl, 0], y[:, sl]),
                    lambda: eng.tensor_mul(w3, w3, yb),
                    lambda: eng.tensor_sub(w3, w3, m2b),
                    lambda: eng_dma.dma_start(out=wd, in_=w3),
                ]
                if pop:
                    ops.append(lambda: head_state.pop(h))
                pending_ops.extend((None, f) for f in ops)

            # ---- emission: one flat (head, chunk, group) position stream.
            # The score stream runs LOOKAHEAD groups ahead of the AV stream
            # (which itself lags a full chunk), so a PSUM-tag refill is
            # already near the front of the PE queue when ScalarE frees the
            # tag -- otherwise the tag round-trip (exp -> refill queued
            # behind an AV range -> exp) paces the pipeline. ----
            load_head(0)
            nsteps = n_heads * nq
            total = nsteps * ngrp
            last = n_heads - 1
            for p in range(-LOOKAHEAD, total + ngrp):
                e = p + LOOKAHEAD  # score event index
                if e < total:
                    step_s, g_s = divmod(e, ngrp)
                    h, qc = divmod(step_s, nq)
                    if g_s == 0:
                        if qc == nq - 2 and h + 1 < n_heads:
                            load_head(h + 1)  # prefetch next head's tensors
                        head_state[h][("umap", qc)] = {}
                    score_group(h, qc, g_s)
                a = p - ngrp  # av event index
                if a >= 0:
                    step_a, g_a = divmod(a, ngrp)
                    ph, pqc = divmod(step_a, nq)
                    av_chunk(ph, pqc, m_ranges[g_a])
                    if g_a == ngrp - 1:
                        head_state[ph].pop(("umap", pqc))
                        norm_chunk(ph, pqc)
                        if ph < last:
                            if pqc == 2 and ph > 0:
                                finish_head_a(ph - 1, 0, nqt)
                            if pqc == 3 and ph > 0:
                                finish_head_b(ph - 1, 0, nqt)
                        else:
                            # last head: finish incrementally so the tail
                            # after the final AV drain is short
                            if pqc == 0:
                                finish_head_a(ph - 1, 0, nqt)
                            if pqc == 1:
                                finish_head_b(ph - 1, 0, nqt)
                            if pqc == 2:
                                q_mid = nqt - tpq
                                finish_head_a(last, 0, q_mid)
                                finish_head_b(last, 0, q_mid, pop=False)
                pump(4)
            # tail: only the last chunk's qtiles remain; ScalarE is idle
            # now, so its Ln/Exp rsqrt (one table switch) beats Newton
            q_mid = nqt - tpq
            finish_head_a(last, q_mid, nqt, on_scalar=TAIL_SC_RSQRT)
            finish_head_b(last, q_mid, nqt, eng=nc.vector)
            pump(len(pending_ops))

    nc.compile()
    return nc


_PROGRAM_CACHE = {}


def _get_program():
    key = (HEADS_PER_CORE, S)
    if key not in _PROGRAM_CACHE:
        _PROGRAM_CACHE[key] = build_program()
    return _PROGRAM_CACHE[key]


def shard_inputs(query, key, value, lambda_params):
    """Full [B,H,S,D] inputs -> per-core input maps (host-side prep)."""
    q = np.asarray(query, dtype=np.float32).reshape(B * H, S, D)
    k = np.asarray(key, dtype=np.float32).reshape(B * H, S, D)
    v = np.asarray(value, dtype=np.float32).reshape(B * H, S, D)
    lam = np.asarray(lambda_params, dtype=np.float32)
    lam_full = np.tile(lam, B)  # pair i = (b=i//H, h=i%H) -> lambda[i%H]
    in_maps = []
    for c in range(N_CORES):
        sl = slice(c * HEADS_PER_CORE, (c + 1) * HEADS_PER_CORE)
        qt = q[sl].transpose(0, 2, 1).astype(np.float16)
        kt = k[sl].transpose(0, 2, 1).astype(np.float16)
        in_maps.append({
            "qt": np.ascontiguousarray(np.concatenate([qt, qt], axis=1)),
            "kt": np.ascontiguousarray(np.concatenate([kt, kt], axis=1)),
            "v": np.ascontiguousarray(v[sl]).astype(np.float16),
            "lam": np.ascontiguousarray(lam_full[sl].reshape(-1, 1)),
        })
    return in_maps


def kernel(query, key, value, lambda_params, trace=False):
    nc = _get_program()
    in_maps = shard_inputs(query, key, value, lambda_params)
    res = run_bass_kernel_spmd(nc, in_maps, core_ids=list(range(N_CORES)),
                               trace=trace)
    out = np.concatenate([r["out"] for r in res.results], axis=0)
    out = out.reshape(B, H, S, D).astype(np.float32)
    if trace:
        kernel.last_exec_time_ns = res.exec_time_ns
        kernel.last_results = res
    return out


# revision 42
# speedup vs baseline: 1.0352x; 1.0352x over previous
"""Differential attention kernel for Trainium2 (8 NeuronCores, SPMD).

Math per (batch, head):
    q1,q2 / k1,k2 = halves of head_dim (D=64 -> d2=32)
    a_i = softmax(q_i @ k_i^T / sqrt(d2))        (i = 1,2)
    out = (a1 - lam*a2) @ V, then per-(q) groupnorm over D, scaled by (1-0.8).

Design (per core: 4 of the 32 (b,h) pairs). The kernel is ScalarE-bound
(~34M exp evaluations/core at 128 lanes x 1.2 GHz ~= 219us pure), so the
structure keeps ScalarE saturated, offloads a slice of the exp work to the
DVE, and keeps everything else off the critical path.
  - Q/K/V cast to fp16 on host (single-pass PE matmul). Host ships Q^T/K^T
    twice ([2D, s]) so 4 score matmuls (K=32) run concurrently in the four
    32-row PE row groups via tile_position.
  - Scores computed transposed: S^T[k, q] units [128, 512] packed into
    alternating 4-bank/2-bank PSUM tiles; each tile drained by ONE ScalarE
    exp (FD 2048/1024) to amortize the ~293ns per-instruction overhead.
    exp needs no max-subtraction: scores ~ N(0,1).
  - Tag patterns alternate by chunk parity so every same-tag PSUM reuse is
    >= 2 groups apart, and score groups are emitted LOOKAHEAD groups ahead
    of the AV stream, so a freed PSUM tag's refill is already at the front
    of the PE queue -- otherwise the exp -> refill -> exp tag round-trip
    paces the pipeline (~3.5us/chunk slower).
    (An X_DVE path that offloads units to a DVE bit-splice exp2 exists but
    is off: every placement ran into the ~3us single-buffered-tag reuse
    deadline vs the DVE op-chain latency and lost 30-100us.)
  - U^T = exp(S^T) fp16 in SBUF; AV matmul lhsT = [V | ones] (M=65) so row
    64 accumulates softmax row-sums r for free. AV chunks pipeline in one
    flat (head, q-chunk) stream across head boundaries.
  - O^T[65, q] per 512-q chunk is copied fp16 to SBUF (pad rows zeroed once
    in static staging tiles); DMA xbar transpose flips [80, 512] ->
    [128, 4, 80] (col 64 of each block = r).
  - Combine exploits scale invariance of groupnorm: instead of
    W = O1*r1inv - lam*O2*r2inv, compute W'' = O1u - (lam*r1/r2)*O2u = r1*W
    (one fused scalar_tensor_tensor per q-tile; batched reciprocals). The
    only difference vs the reference is EPS seen as EPS/r1^2 ~ 0 inside
    rsqrt(var+EPS) -- a ~2e-3 relative effect since var >> EPS here.
  - rstd = (1-lam0)*rsqrt(var+EPS): integer-magic seed on the DVE, then the
    Newton iteration and the final (W - mean)*rstd rescale run on GPSIMD
    (otherwise idle, own queue) -- keeping the DVE FIFO clear is critical,
    since the AV PSUM copy-outs gate the next pav accumulation through the
    PE's in-order queue. The last head finishes incrementally, its final
    q-tiles via ScalarE ln/exp (table switch is free once exps are done),
    so the post-loop tail stays short. A strided store DMA per range.
"""

import math
import numpy as np

import concourse.bass as bass
import concourse.tile as tile
from concourse import bacc, mybir
from concourse.bass_utils import run_bass_kernel_spmd

F32 = mybir.dt.float32
F16 = mybir.dt.float16
I32 = mybir.dt.int32
AF = mybir.ActivationFunctionType
ALU = mybir.AluOpType

B, H, S, D = 2, 16, 2048, 64
D2 = D // 2
N_CORES = 8
HEADS_PER_CORE = (B * H) // N_CORES  # 4
LAMBDA_INIT = 0.8
EPS = 1e-5
SCALE = 1.0 / math.sqrt(D2)
LOG2E = math.log2(math.e)

QC = 512           # q chunk (one PSUM bank of fp32)
KTILE = 128        # k tile (partition dim)
OROWS = 80         # O^T rows padded to xbar 16-row granularity (65 -> 80)

# --- tunables ---
X_DVE = 0          # score units per chunk exp'd on the DVE
AV_BUFS = 2
GP_FINISH = False   # final rescale on GPSIMD instead of DVE
SC_RSQRT = False    # rstd via ScalarE ln/exp instead of DVE Newton
U_BUFS = 30
LOOKAHEAD = 4      # score groups emitted ahead of the AV stream (PE queue
                   # position of a PSUM-tag refill when ScalarE frees it)
NEWTON_ITERS = 4   # rsqrt Newton steps (err 1.5e^2 per step)
TAIL_SC_RSQRT = True  # last head's rsqrt on the (by then idle) ScalarE

# 2^f quadratic on [-0.5, 0.5], minimax relative error ~1.7e-3
EXP2_C0 = 1.0004428354418449
EXP2_C1 = 0.7034397194807023
EXP2_C2 = 0.23842415021588936
MAGIC = float(1.5 * 2 ** 23)   # fp32 round-to-int magic


def chunk_plan(nkt, x_dve):
    """Per-chunk-parity group plans. Each group is one PSUM tile (ps4 = 4
    banks / ps2 = 2 banks) filled by the PE; groups with sc units are
    drained by one ScalarE exp, dve units by the DVE's bit-splice exp2.
    Tag patterns alternate by chunk parity so every same-tag reuse
    (including across chunk boundaries) is >= 2 groups apart -- a 1-group
    reuse serializes PE fill against ScalarE drain once per chunk. The
    dve-only group sits mid-chunk, away from the end-of-chunk norm blob,
    so the DVE drains its tile within the 2-group reuse deadline."""
    dve_units = [(nkt - 1 - i, 1) for i in range(x_dve)]  # a2 half
    sc_units = [(kt, h) for kt in range(nkt) for h in (0, 1)
                if (kt, h) not in dve_units]
    plans = []
    for par in range(2):
        if x_dve == 0:
            if par == 0:
                sizes = [4, 2, 4, 2, 4, 2, 4, 2, 4, 2, 2]
                tags = ["ps4", "ps2"] * 5 + ["ps4"]
            else:
                sizes = [2, 4, 2, 4, 2, 4, 2, 4, 2, 4, 2]
                tags = ["ps2", "ps4"] * 5 + ["ps2"]
            dve_at = None
        elif x_dve == 2:
            if par == 0:
                sizes = [4, 2, 4, 2, 4, 0, 4, 2, 4, 2, 2]
                tags = ["ps4", "ps2"] * 5 + ["ps4"]
                dve_at = 5
            else:
                sizes = [2, 4, 2, 4, 0, 4, 2, 4, 2, 4, 2]
                tags = ["ps2", "ps4"] * 5 + ["ps2"]
                dve_at = 4
        else:
            raise ValueError(x_dve)
        assert sum(sizes) == 2 * nkt - x_dve
        groups = []
        pos = 0
        for n in sizes:
            groups.append(sc_units[pos:pos + n])
            pos += n
        dve_in = {dve_at: list(dve_units)} if dve_at is not None else {}
        plans.append((groups, tags, dve_in))
    ngrp = len(plans[0][0])
    def mk_ranges(seq):
        return [seq[round(2 * nkt * g / ngrp):round(2 * nkt * (g + 1) / ngrp)]
                for g in range(ngrp)]
    m_ranges = mk_ranges(
        [kt + half * nkt for kt in range(nkt) for half in (0, 1)])
    m_ranges_il = m_ranges
    # verify all same-tag reuse distances >= 2 across the alternation
    seq = [t for par in (0, 1) for t in plans[par][1]] * 2
    for i in range(len(seq)):
        for j in range(i + 1, len(seq)):
            if seq[j] == seq[i]:
                assert j - i >= 2, (i, j)
                break
    return plans, m_ranges, m_ranges_il


def build_program(n_heads=HEADS_PER_CORE, s=S):
    nq = s // QC          # q chunks per head
    nkt = s // KTILE      # k tiles per head
    nqt = s // 128        # q tiles (norm phase)
    tpq = QC // 128       # q tiles per chunk
    plans, m_ranges, m_ranges_il = chunk_plan(nkt, X_DVE)
    ngrp = len(plans[0][0])

    nc = bacc.Bacc("TRN2", target_bir_lowering=False, debug=False,
                   num_devices=N_CORES)
    qt_d = nc.dram_tensor("qt", [n_heads, 2 * D, s], F16, kind="ExternalInput")
    kt_d = nc.dram_tensor("kt", [n_heads, 2 * D, s], F16, kind="ExternalInput")
    v_d = nc.dram_tensor("v", [n_heads, s, D], F16, kind="ExternalInput")
    lam_d = nc.dram_tensor("lam", [n_heads, 1], F32, kind="ExternalInput")
    out_d = nc.dram_tensor("out", [n_heads, s, D], F32, kind="ExternalOutput")

    gp = nc.gpsimd

    with tile.TileContext(nc) as tc:
        with (
            tc.tile_pool(name="consts", bufs=1) as consts,
            tc.tile_pool(name="qk", bufs=2) as qk_pool,
            tc.tile_pool(name="vx", bufs=2) as vx_pool,
            tc.tile_pool(name="lamp", bufs=2) as lam_pool,
            tc.tile_pool(name="u", bufs=U_BUFS) as u_pool,
            tc.tile_pool(name="ud", bufs=2 * X_DVE + 2) as ud_pool,
            tc.tile_pool(name="dv", bufs=2) as dv_pool,
            tc.tile_pool(name="o", bufs=1) as o_pool,
            tc.tile_pool(name="tr", bufs=6) as tr_pool,
            tc.tile_pool(name="w", bufs=3) as w_pool,
            tc.tile_pool(name="stats", bufs=3) as stats_pool,
            tc.tile_pool(name="small", bufs=8) as small_pool,
            tc.tile_pool(name="ps_sc", bufs=1, space="PSUM") as ps_scores,
            tc.tile_pool(name="ps_av", bufs=AV_BUFS, space="PSUM") as ps_av,
        ):
            eps_ap = consts.tile([128, 1], F32)
            nc.vector.memset(eps_ap, EPS)
            bexp_ap = consts.tile([128, 1], F32)
            nc.vector.memset(bexp_ap, math.log(1.0 - LAMBDA_INIT))
            zero_ap = consts.tile([128, 1], F32)
            nc.vector.memset(zero_ap, 0.0)
            consts_c15 = consts.tile([128, 16], F32)
            nc.vector.memset(consts_c15, 1.5)
            consts_cfin = consts.tile([128, 16], F32)
            nc.vector.memset(consts_cfin,
                             (1.0 - LAMBDA_INIT) / math.sqrt(2.0))
            # prefetch the exp table set while the first DMAs run
            warm = consts.tile([128, 1], F32)
            if SC_RSQRT:
                nc.scalar.activation(warm, eps_ap, AF.Ln, bias=eps_ap)
                nc.scalar.activation(warm, warm, AF.Exp)
            else:
                nc.scalar.activation(warm, eps_ap, AF.Exp)

            # Two static sets of O^T staging tiles (heads alternate) so the
            # xbar-transpose pad rows 65..79 are zeroed exactly once.
            o_static = []
            for par in range(2):
                o1s = o_pool.tile([OROWS, s], F16, tag=f"o1_{par}")
                o2s = o_pool.tile([OROWS, s], F16, tag=f"o2_{par}")
                nc.gpsimd.memset(o1s[D:OROWS, :], 0.0)
                nc.gpsimd.memset(o2s[D:OROWS, :], 0.0)
                o_static.append((o1s, o2s))

            head_state = {}
            pending_ops = []  # (tag, fn) FIFO; tag marks a DVE-exp unit

            def pump(n):
                for _ in range(min(n, len(pending_ops))):
                    pending_ops.pop(0)[1]()

            def pump_until_emitted(tag):
                """Emit queued ops up to and including the last op of `tag`
                (its consumer is about to be emitted)."""
                while any(t == tag for t, _ in pending_ops):
                    pending_ops.pop(0)[1]()

            def load_head(h):
                qt_sb = qk_pool.tile([2 * D, s], F16, tag="qt")
                kt_sb = qk_pool.tile([2 * D, s], F16, tag="kt")
                if h == 0:
                    # finely chunked so the first score group starts early
                    for tsb, td, c0, c1 in (
                        (kt_sb, kt_d, 0, 128), (qt_sb, qt_d, 0, 512),
                        (kt_sb, kt_d, 128, 512), (kt_sb, kt_d, 512, 1024),
                        (kt_sb, kt_d, 1024, 2048), (qt_sb, qt_d, 512, 2048),
                    ):
                        nc.sync.dma_start(out=tsb[:, c0:c1],
                                          in_=td[h][:, c0:c1])
                else:
                    nc.sync.dma_start(out=qt_sb, in_=qt_d[h])
                    nc.sync.dma_start(out=kt_sb, in_=kt_d[h])
                vx = vx_pool.tile([128, nkt, D + 1], F16, tag="vx")
                nc.sync.dma_start(
                    out=vx[:, :, 0:D],
                    in_=v_d[h].rearrange("(t p) d -> p t d", p=128),
                )
                nc.vector.memset(vx[:, :, D : D + 1], 1.0)
                lamneg = lam_pool.tile([128, 1], F32, tag="lam")
                nc.sync.dma_start(out=lamneg, in_=lam_d[h].to_broadcast((128, 1)))
                nc.vector.tensor_scalar_mul(lamneg, lamneg, -1.0)
                o1, o2 = o_static[h % 2]
                w_head = w_pool.tile([128, nqt, D], F32, tag="w")
                head_state[h] = dict(qt=qt_sb, kt=kt_sb, vx=vx, lamneg=lamneg,
                                     o1=o1, o2=o2, w=w_head, mv=None)

            def queue_dve_exp(h, qc, kt, half, ps_src):
                """One score unit exp'd on the DVE from a PSUM slice: exact
                2^n bit-splice times quadratic 2^f. Ops are queued and
                dribbled so they never form a DVE-FIFO block ahead of the
                AV copy-outs."""
                st = head_state[h]
                ud = ud_pool.tile([128, QC], F16, tag="ud")
                bu = dv_pool.tile([128, QC], F32, tag="bu")
                bn = dv_pool.tile([128, QC], F32, tag="bn")
                be = dv_pool.tile([128, QC], F32, tag="be")
                bf = dv_pool.tile([128, QC], F32, tag="bf")
                bp = dv_pool.tile([128, QC], F32, tag="bp")
                a = SCALE * LOG2E
                for f in [
                    # u = t*a + MAGIC  (rounds t*a to int in low mantissa)
                    lambda: nc.vector.tensor_scalar(
                        out=bu, in0=ps_src, scalar1=a, scalar2=MAGIC,
                        op0=ALU.mult, op1=ALU.add),
                    # n = u - MAGIC (exact)
                    lambda: nc.vector.tensor_scalar(
                        out=bn, in0=bu, scalar1=MAGIC, scalar2=None,
                        op0=ALU.subtract),
                    # 2^n bits = (u_bits << 23) + (127 << 23); walrus forbids
                    # mixing bitwise and arith op classes in one instruction
                    lambda: nc.vector.tensor_scalar(
                        out=be[:].bitcast(I32), in0=bu[:].bitcast(I32),
                        scalar1=23, scalar2=None,
                        op0=ALU.logical_shift_left),
                    lambda: nc.vector.tensor_scalar(
                        out=be[:].bitcast(I32), in0=be[:].bitcast(I32),
                        scalar1=(127 << 23), scalar2=None, op0=ALU.add),
                    # f = t*a - n  in [-0.5, 0.5]
                    lambda: nc.vector.scalar_tensor_tensor(
                        out=bf, in0=ps_src, scalar=a, in1=bn,
                        op0=ALU.mult, op1=ALU.subtract),
                    # p = c2*f + c1
                    lambda: nc.vector.tensor_scalar(
                        out=bp, in0=bf, scalar1=EXP2_C2, scalar2=EXP2_C1,
                        op0=ALU.mult, op1=ALU.add),
                    # p = p*f
                    lambda: nc.vector.tensor_mul(bp, bp, bf),
                    # u_out = (p + c0) * 2^n
                    lambda: nc.vector.scalar_tensor_tensor(
                        out=ud, in0=bp, scalar=EXP2_C0, in1=be,
                        op0=ALU.add, op1=ALU.mult),
                ]:
                    f()
                st[("umap", qc)][half * nkt + kt] = ud[:, :]

            def score_group(h, qc, g):
                """One group of score units -> one PSUM tile -> one exp
                (or, for the dve-only group, eight queued DVE ops)."""
                st = head_state[h]
                qt_sb, kt_sb = st["qt"], st["kt"]
                groups, tags, dve_in = plans[(h * nq + qc + 1) % 2]
                units = groups[g]
                dunits = dve_in.get(g, ())
                n = len(units)
                nb = 4 if tags[g] == "ps4" else 2
                ps = ps_scores.tile([128, nb * QC], F32, tag=tags[g])
                for i, (kt, half) in enumerate(list(units) + list(dunits)):
                    j = 2 * kt + half
                    rb = (j % 4) * D2
                    dsl = slice(rb, rb + D2)
                    nc.tensor.matmul(
                        ps[:, i * QC : (i + 1) * QC],
                        kt_sb[dsl, kt * KTILE : (kt + 1) * KTILE],
                        qt_sb[dsl, qc * QC : (qc + 1) * QC],
                        start=True, stop=True,
                        tile_position=(rb, 0),
                    )
                if n:
                    u = u_pool.tile([128, 4 * QC], F16, tag="u")
                    nc.scalar.activation(
                        u[:, 0 : n * QC], ps[:, 0 : n * QC], AF.Exp,
                        scale=SCALE)
                    for i, (kt, half) in enumerate(units):
                        st[("umap", qc)][half * nkt + kt] = (
                            u[:, i * QC : (i + 1) * QC])
                for i, (kt, half) in enumerate(dunits):
                    sl = slice((n + i) * QC, (n + i + 1) * QC)
                    queue_dve_exp(h, qc, kt, half, ps[:, sl])

            def av_chunk(h, qc, ms):
                """AV matmuls for m in ms; m = half*nkt + kt."""
                st = head_state[h]
                vx = st["vx"]
                umap = st[("umap", qc)]
                for m in ms:
                    half, kt = divmod(m, nkt)
                    if kt == 0:
                        pav_new = ps_av.tile([D + 1, QC], F32, tag="pav")
                        st[("pav", qc, half)] = pav_new
                    pav = st[("pav", qc, half)]
                    nc.tensor.matmul(
                        pav, vx[:, kt, :], umap.pop(m),
                        start=(kt == 0), stop=(kt == nkt - 1),
                    )
                    if kt == nkt - 1:
                        o_sb = st["o2"] if half else st["o1"]
                        nc.vector.tensor_copy(
                            o_sb[0 : D + 1, qc * QC : (qc + 1) * QC], pav)
                        del st[("pav", qc, half)]

            def norm_chunk(h, qc):
                """DMA-xbar transpose of one q chunk, scale-free combine,
                stats. W'' = O1u - (lam*r1/r2)*O2u = r1*W; groupnorm is
                invariant to the per-row r1 factor (up to EPS/r1^2 ~ 0)."""
                st = head_state[h]
                o1, o2, lamneg = st["o1"], st["o2"], st["lamneg"]
                if st["mv"] is None:
                    mv_new = stats_pool.tile([128, nqt, 2], F32, tag="mv")
                    st["mv"] = mv_new
                mv = st["mv"]
                csl = slice(qc * QC, (qc + 1) * QC)
                tr1 = tr_pool.tile([128, tpq, OROWS], F16, tag="tr1")
                nc.sync.dma_start_transpose(tr1, o1[:, csl])
                tr2 = tr_pool.tile([128, tpq, OROWS], F16, tag="tr2")
                nc.sync.dma_start_transpose(tr2, o2[:, csl])
                rho = small_pool.tile([128, tpq], F32, tag="rho")
                nc.vector.reciprocal(rho, tr2[:, :, D])
                # rho = -lam * r1 / r2
                nc.vector.tensor_mul(rho, rho, tr1[:, :, D])
                nc.vector.tensor_scalar_mul(rho, rho, lamneg)
                for t in range(tpq):
                    qt_i = qc * tpq + t
                    w = st["w"][:, qt_i, :]
                    nc.vector.scalar_tensor_tensor(
                        out=w, in0=tr2[:, t, 0:D],
                        scalar=rho[:, t : t + 1], in1=tr1[:, t, 0:D],
                        op0=ALU.mult, op1=ALU.add,
                    )
                    s6 = small_pool.tile([128, 6], F32, tag="s6")
                    nc.vector.bn_stats(out=s6, in_=w)
                    nc.vector.bn_aggr(out=mv[:, qt_i, :], in_=s6)

            def finish_head_a(h, q0, q1, on_scalar=False):
                """y[:, q0:q1] = (1-lam0) * rsqrt(var + EPS): either two
                ScalarE activations exp(-0.5*ln(var+EPS) + ln(1-lam0)) --
                used only when ScalarE has gone idle (tail), since Ln lives
                in a different table set than Exp -- or a DVE Newton
                iteration with an exponent-only integer seed, dribbled via
                the pending queue."""
                st = head_state[h]
                mv = st["mv"]
                if "y" not in st:
                    y_new = stats_pool.tile([128, nqt], F32, tag="y")
                    st["y"] = y_new
                y = st["y"]
                sl = slice(q0, q1)
                if on_scalar:
                    lnv = stats_pool.tile([128, nqt], F32, tag="lnv")
                    yt = stats_pool.tile([128, nqt], F32, tag="yt")
                    st["y"] = yt
                    nc.scalar.activation(lnv[:, sl], mv[:, sl, 1], AF.Ln,
                                         bias=eps_ap)
                    nc.scalar.activation(yt[:, sl], lnv[:, sl], AF.Exp,
                                         bias=bexp_ap, scale=-0.5)
                    return
                # DVE Newton rsqrt (exponent seed + NEWTON_ITERS steps)
                x = stats_pool.tile([128, nqt], F32, tag="x")
                t = stats_pool.tile([128, nqt], F32, tag="t")
                c15 = consts_c15
                cfin = consts_cfin
                xs, ys, ts = x[:, sl], y[:, sl], t[:, sl]
                ops = [
                    lambda: nc.vector.tensor_scalar(
                        out=xs, in0=mv[:, sl, 1], scalar1=EPS,
                        scalar2=0.25, op0=ALU.add, op1=ALU.mult),
                    lambda: nc.vector.tensor_scalar(
                        out=ys.bitcast(I32), in0=xs.bitcast(I32),
                        scalar1=23, scalar2=None,
                        op0=ALU.logical_shift_right),
                    lambda: nc.vector.tensor_scalar(
                        out=ys.bitcast(I32), in0=ys.bitcast(I32),
                        scalar1=-1, scalar2=380, op0=ALU.mult, op1=ALU.add),
                    lambda: nc.vector.tensor_scalar(
                        out=ys.bitcast(I32), in0=ys.bitcast(I32),
                        scalar1=1, scalar2=None,
                        op0=ALU.logical_shift_right),
                    lambda: nc.vector.tensor_scalar(
                        out=ys.bitcast(I32), in0=ys.bitcast(I32),
                        scalar1=23, scalar2=None,
                        op0=ALU.logical_shift_left),
                ]
                for _ in range(NEWTON_ITERS):
                    ops += [
                        lambda: gp.tensor_mul(ts, ys, ys),
                        lambda: gp.tensor_mul(ts, ts, xs),
                        lambda: gp.tensor_sub(ts, c15[:, sl], ts),
                        lambda: gp.tensor_mul(ys, ys, ts),
                    ]
                ops.append(lambda: gp.tensor_mul(ys, ys, cfin[:, sl]))
                pending_ops.extend((None, f) for f in ops)

            def finish_head_b(h, q0, q1, pop=True, eng=None):
                st = head_state[h]
                mv, y = st["mv"], st["y"]
                if eng is not None or "m2" not in st:
                    tag2 = "m2t" if eng is not None else "m2"
                    m2_new = stats_pool.tile([128, nqt], F32, tag=tag2)
                    st["m2"] = m2_new
                m2 = st["m2"]
                nq_ = q1 - q0
                sl = slice(q0, q1)
                w3 = st["w"][:, sl, :]
                yb = y[:, sl][:, :, None].broadcast_to((128, nq_, D))
                m2b = m2[:, sl][:, :, None].broadcast_to((128, nq_, D))
                wd = out_d[h].rearrange("(t p) d -> p t d", p=128)[:, sl, :]
                eng_dma = nc.sync
                eng = eng or gp
                ops = [
                    lambda: eng.tensor_mul(m2[:, sl], mv[:, sl, 0], y[:, sl]),
                    lambda: eng.tensor_mul(w3, w3, yb),
                    lambda: eng.tensor_sub(w3, w3, m2b),
                    lambda: eng_dma.dma_start(out=wd, in_=w3),
                ]
                if pop:
                    ops.append(lambda: head_state.pop(h))
                pending_ops.extend((None, f) for f in ops)

            # ---- emission: one flat (head, chunk, group) position stream.
            # The score stream runs LOOKAHEAD groups ahead of the AV stream
            # (which itself lags a full chunk), so a PSUM-tag refill is
            # already near the front of the PE queue when ScalarE frees the
            # tag -- otherwise the tag round-trip (exp -> refill queued
            # behind an AV range -> exp) paces the pipeline. ----
            load_head(0)
            nsteps = n_heads * nq
            total = nsteps * ngrp
            last = n_heads - 1
            for p in range(-LOOKAHEAD, total + ngrp):
                e = p + LOOKAHEAD  # score event index
                if e < total:
                    step_s, g_s = divmod(e, ngrp)
                    h, qc = divmod(step_s, nq)
                    if g_s == 0:
                        if qc == nq - 2 and h + 1 < n_heads:
                            load_head(h + 1)  # prefetch next head's tensors
                        head_state[h][("umap", qc)] = {}
                    score_group(h, qc, g_s)
                a = p - ngrp  # av event index
                if a >= 0:
                    step_a, g_a = divmod(a, ngrp)
                    ph, pqc = divmod(step_a, nq)
                    av_chunk(ph, pqc, m_ranges[g_a])
                    if g_a == ngrp - 1:
                        head_state[ph].pop(("umap", pqc))
                        norm_chunk(ph, pqc)
                        if ph < last:
                            if pqc == 2 and ph > 0:
                                finish_head_a(ph - 1, 0, nqt)
                            if pqc == 3 and ph > 0:
                                finish_head_b(ph - 1, 0, nqt)
                        else:
                            # last head: finish incrementally so the tail
                            # after the final AV drain is short
                            if pqc == 0:
                                finish_head_a(ph - 1, 0, nqt)
                            if pqc == 1:
                                finish_head_b(ph - 1, 0, nqt)
                            if pqc == 2:
                                q_mid = nqt - tpq
                                finish_head_a(last, 0, q_mid)
                                finish_head_b(last, 0, q_mid, pop=False)
                pump(4)
            # tail: only the last chunk's qtiles remain; ScalarE is idle
            # now, so its Ln/Exp rsqrt (one table switch) beats Newton
            q_mid = nqt - tpq
            finish_head_a(last, q_mid, nqt, on_scalar=TAIL_SC_RSQRT)
            finish_head_b(last, q_mid, nqt, eng=nc.vector)
            pump(len(pending_ops))

    nc.compile()
    return nc


_PROGRAM_CACHE = {}


def _get_program():
    key = (HEADS_PER_CORE, S)
    if key not in _PROGRAM_CACHE:
        _PROGRAM_CACHE[key] = build_program()
    return _PROGRAM_CACHE[key]


def shard_inputs(query, key, value, lambda_params):
    """Full [B,H,S,D] inputs -> per-core input maps (host-side prep)."""
    q = np.asarray(query, dtype=np.float32).reshape(B * H, S, D)
    k = np.asarray(key, dtype=np.float32).reshape(B * H, S, D)
    v = np.asarray(value, dtype=np.float32).reshape(B * H, S, D)
    lam = np.asarray(lambda_params, dtype=np.float32)
    lam_full = np.tile(lam, B)  # pair i = (b=i//H, h=i%H) -> lambda[i%H]
    in_maps = []
    for c in range(N_CORES):
        sl = slice(c * HEADS_PER_CORE, (c + 1) * HEADS_PER_CORE)
        qt = q[sl].transpose(0, 2, 1).astype(np.float16)
        kt = k[sl].transpose(0, 2, 1).astype(np.float16)
        in_maps.append({
            "qt": np.ascontiguousarray(np.concatenate([qt, qt], axis=1)),
            "kt": np.ascontiguousarray(np.concatenate([kt, kt], axis=1)),
            "v": np.ascontiguousarray(v[sl]).astype(np.float16),
            "lam": np.ascontiguousarray(lam_full[sl].reshape(-1, 1)),
        })
    return in_maps


def kernel(query, key, value, lambda_params, trace=False):
    nc = _get_program()
    in_maps = shard_inputs(query, key, value, lambda_params)
    res = run_bass_kernel_spmd(nc, in_maps, core_ids=list(range(N_CORES)),
                               trace=trace)
    out = np.concatenate([r["out"] for r in res.results], axis=0)
    out = out.reshape(B, H, S, D).astype(np.float32)
    if trace:
        kernel.last_exec_time_ns = res.exec_time_ns
        kernel.last_results = res
    return out
